# revision 1
# baseline (speedup 1.0000x reference)
"""Trainium2 Bass kernel for the contextual channel-attention transformer block.

Contract: kernel(**inputs) takes the FULL unsharded inputs
(x: (8,512,64,64) f32, Wq/Wk/Wv: (512,512) f32, gamma: (1,) f32) and
returns the FULL (8,512,64,64) f32 output.  Internally the batch is
data-parallel across 8 NeuronCores (one batch element per core).

Per-core algorithm (all bf16 matmuls, fp32 PSUM accumulation):
  Gx   = X @ X.T                     (C x C spatial Gram, 128 MMs)
  M3q  = Gx @ Wq.T, M3k = Gx @ Wk.T  (32 MMs)
  G^T  = Wk @ M3q  = (Q @ K.T).T     (16 MMs)
  |Q_c|^2 = diag(Wq Gx Wq.T) = colsum(Wq.T o M3q)   (cheap)
  cos -> col-max -> temperature -> softmax: free-axis ops on G^T[d, c]
  A^T  = Wv.T @ Msm^T = (Msm @ Wv).T (16 MMs)
  out  = A @ X                       (128 MMs)
  y    = x + (gamma / rowsum(Msm)) * out   (folded into per-partition scale)
"""

import os
import sys

for _p in ("/opt/trn_rl_repo", "/root/.axon_site/_ro/trn_rl_repo"):
    if os.path.isdir(_p) and _p not in sys.path:
        sys.path.insert(0, _p)

import ml_dtypes
import numpy as np

import concourse.bass as bass
import concourse.tile as tile
from concourse import bacc, bass_utils, mybir

# Problem constants (hardcoded; kernel.py must be self-contained).
B, C, HH, WW = 8, 512, 64, 64
N = HH * WW          # 4096 spatial positions
G = C // 128         # 4 channel groups of 128
N1 = N // 128        # 32 Gram chunks (128 spatial each)
NJ = N // 512        # 8 output chunks (512 spatial each)
EPS = 1e-6
INV_H = 4.0          # 1 / 0.25 temperature
FP32 = mybir.dt.float32
BF16 = mybir.dt.bfloat16

_CACHE = {}


def _warm(nc, pool, src_ap, k):
    scr = pool.tile([1, 1], mybir.dt.float32, tag="scr", bufs=2,
                    name=f"scr{k}")
    nc.tensor.matmul(scr[:], src_ap, src_ap, start=True, stop=True)


def _phase2_chunk(nc, tc, ps2, opool, at_sb, xh_t, fcols, xf_t, y_v, j):
    FP32 = mybir.dt.float32
    ADD = mybir.AluOpType.add
    Copy = mybir.ActivationFunctionType.Copy
    ofin = opool.tile([128, G, 512], FP32, tag="ofin", bufs=3,
                      name=f"ofin{j}")
    for cg in range(G):
        o_ps = ps2.tile([128, 512], FP32, tag="o_ps", bufs=6,
                        name=f"o_ps{j}_{cg}")
        for eg in range(G):
            nc.tensor.matmul(
                o_ps[:], at_sb[:, eg, cg * 128:(cg + 1) * 128],
                xh_t[j][:, eg, :],
                start=(eg == 0), stop=(eg == G - 1))
        osc = opool.tile([128, 512], FP32, tag="osc", bufs=4,
                         name=f"osc{j}_{cg}")
        nc.scalar.activation(osc[:], o_ps[:], Copy, scale=fcols[cg][:])
        nc.vector.tensor_tensor(ofin[:, cg, :], osc[:], xf_t[:, cg, :],
                                op=ADD)
    if j == NJ - 1:
        for cg in range(G):
            nc.sync.dma_start(y_v[:, cg, j * 512:(j + 1) * 512],
                              ofin[:, cg, :])
    else:
        nc.sync.dma_start(y_v[:, :, j * 512:(j + 1) * 512], ofin[:])


def _build_nc():
    nc = bacc.Bacc("TRN2", target_bir_lowering=False)

    xt_d = nc.dram_tensor("xt", [N, C], BF16, kind="ExternalInput")   # x^T
    xh_d = nc.dram_tensor("xh", [C, N], BF16, kind="ExternalInput")
    xf_d = nc.dram_tensor("xf", [C, N], FP32, kind="ExternalInput")
    wqt_d = nc.dram_tensor("wqt", [C, C], BF16, kind="ExternalInput")  # Wq^T
    wkt_d = nc.dram_tensor("wkt", [C, C], BF16, kind="ExternalInput")  # Wk^T
    wvo_d = nc.dram_tensor("wvo", [C, C], BF16, kind="ExternalInput")  # Wv
    gcol_d = nc.dram_tensor("gamma_col", [128, 1], FP32, kind="ExternalInput")
    ocol_d = nc.dram_tensor("ones_col", [128, 1], BF16, kind="ExternalInput")
    orow_d = nc.dram_tensor("ones_row", [1, C], BF16, kind="ExternalInput")
    y_d = nc.dram_tensor("y", [C, N], FP32, kind="ExternalOutput")

    xt_v = xt_d.ap().rearrange("(i p) c -> p i c", p=128)    # [128, N1, C]
    xh_v = xh_d.ap().rearrange("(g p) n -> p g n", p=128)    # [128, G, N]
    xf_v = xf_d.ap().rearrange("(g p) n -> p g n", p=128)
    wq_v = wqt_d.ap().rearrange("(g p) o -> p g o", p=128)   # [128, G, C]
    wk_v = wkt_d.ap().rearrange("(g p) o -> p g o", p=128)
    wv_v = wvo_d.ap().rearrange("(g p) o -> p g o", p=128)
    y_v = y_d.ap().rearrange("(g p) n -> p g n", p=128)

    MUL = mybir.AluOpType.mult
    ADD = mybir.AluOpType.add
    MIN = mybir.AluOpType.min
    AX = mybir.AxisListType.X
    Exp = mybir.ActivationFunctionType.Exp
    Ln = mybir.ActivationFunctionType.Ln
    Copy = mybir.ActivationFunctionType.Copy

    with tile.TileContext(nc) as tc:
        with (
            tc.tile_pool(name="consts", bufs=1) as cpool,
            tc.tile_pool(name="weights", bufs=1) as wpool,
            tc.tile_pool(name="xt", bufs=NJ) as xtpool,
            tc.tile_pool(name="xh", bufs=NJ) as xhpool,
            tc.tile_pool(name="gram", bufs=1) as gpool,
            tc.tile_pool(name="small", bufs=2) as spool,
            tc.tile_pool(name="mid", bufs=3) as mpool,
            tc.tile_pool(name="msm", bufs=1) as msmpool,
            tc.tile_pool(name="ph2", bufs=2) as p2pool,
            tc.tile_pool(name="outs", bufs=4) as opool,
        ):
            # ---- input DMAs (xt first: Gx depends only on it) ------------
            xt0 = []
            for i in range(G):
                t = xtpool.tile([128, 1, C], BF16, tag="xt0", bufs=G,
                                name=f"xt0_{i}")
                nc.sync.dma_start(t[:], xt_v[:, i:i + 1, :])
                xt0.append(t)
            xt_t = []
            for jj in range(1, NJ):
                t = xtpool.tile([128, G, C], BF16, tag="xt", bufs=NJ - 1,
                                name=f"xt{jj}")
                nc.sync.dma_start(t[:], xt_v[:, jj * G:(jj + 1) * G, :])
                xt_t.append(t)

            def xt_chunk(i):
                return xt0[i][:, 0, :] if i < G else xt_t[i // G - 1][:, i % G, :]

            ones_col = cpool.tile([128, 1], BF16, tag="ones_col")
            nc.sync.dma_start(ones_col[:], ocol_d.ap())
            ones_row = cpool.tile([1, C], BF16, tag="ones_row")
            nc.sync.dma_start(ones_row[:], orow_d.ap())
            gamma_col = cpool.tile([128, 1], FP32, tag="gamma_col")
            nc.sync.dma_start(gamma_col[:], gcol_d.ap())

            wq = wpool.tile([128, G, C], BF16, tag="wq")
            wk = wpool.tile([128, G, C], BF16, tag="wk")
            wv = wpool.tile([128, G, C], BF16, tag="wv")
            nc.sync.dma_start(wq[:], wq_v)
            nc.sync.dma_start(wk[:], wk_v)
            nc.sync.dma_start(wv[:], wv_v)

            xh_t = []
            for j in range(NJ):
                t = xhpool.tile([128, G, 512], BF16, tag="xh", name=f"xh{j}")
                nc.sync.dma_start(t[:], xh_v[:, :, j * 512:(j + 1) * 512])
                xh_t.append(t)

            # ---- Gx = X X^T  (PSUM-accumulated over 32 spatial chunks) ---
            gx_sb = gpool.tile([128, G, C], BF16, tag="gx_sb")
            with tc.tile_pool(name="psGx", bufs=1, space="PSUM") as psGx:
                gx_ps = [psGx.tile([128, C], FP32, tag="gx", bufs=G,
                                   name=f"gx{cg}") for cg in range(G)]
                for i in range(N1):
                    lhs_t = xt_chunk(i)
                    for cg in range(G):
                        nc.tensor.matmul(gx_ps[cg][:],
                                         lhs_t[:, cg * 128:(cg + 1) * 128],
                                         lhs_t[:],
                                         start=(i == 0), stop=(i == N1 - 1))
                for cg in range(G):
                    eng = nc.scalar.copy if cg % 2 else nc.vector.tensor_copy
                    eng(gx_sb[:, cg, :], gx_ps[cg][:])

            # ---- M3q = Gx Wq^T, M3k = Gx Wk^T ----------------------------
            m3q = gpool.tile([128, G, C], BF16, tag="m3q")
            m3k = gpool.tile([128, G, C], BF16, tag="m3k")
            with tc.tile_pool(name="psM3", bufs=1, space="PSUM") as psM3:
                for cg in range(G):
                    q_ps = psM3.tile([128, C], FP32, tag="m3q", bufs=G,
                                     name=f"m3q{cg}")
                    k_ps = psM3.tile([128, C], FP32, tag="m3k", bufs=G,
                                     name=f"m3k{cg}")
                    for g in range(G):
                        lhs = gx_sb[:, g, cg * 128:(cg + 1) * 128]
                        nc.tensor.matmul(q_ps[:], lhs, wq[:, g, :],
                                         start=(g == 0), stop=(g == G - 1))
                        nc.tensor.matmul(k_ps[:], lhs, wk[:, g, :],
                                         start=(g == 0), stop=(g == G - 1))
                    nc.scalar.copy(m3q[:, cg, :], q_ps[:])
                    nc.vector.tensor_copy(m3k[:, cg, :], k_ps[:])

            msm = msmpool.tile([128, G, C], BF16, tag="msm")
            at_sb = gpool.tile([128, G, C], BF16, tag="at_sb")
            fcols = []
            with tc.tile_pool(name="psN", bufs=1, space="PSUM") as psN:
                # ---- norms: |Q_c|^2 row, |K_d|^2 columns -----------------
                sqq = psN.tile([1, C], FP32, tag="sqq", name="sqq")
                sqk_ps = [psN.tile([128, 1], FP32, tag="sqk", bufs=G,
                                   name=f"sqk{d}") for d in range(G)]
                for g in range(G):
                    tq = mpool.tile([128, C], BF16, tag="tq")
                    nc.vector.tensor_tensor(tq[:], wq[:, g, :], m3q[:, g, :],
                                            op=MUL)
                    nc.tensor.matmul(sqq[:], ones_col[:], tq[:],
                                     start=(g == 0), stop=(g == G - 1))
                    tk = mpool.tile([128, C], BF16, tag="tk")
                    nc.vector.tensor_tensor(tk[:], wk[:, g, :],
                                            m3k[:, g, :], op=MUL)
                    for dg in range(G):
                        nc.tensor.matmul(sqk_ps[dg][:],
                                         tk[:, dg * 128:(dg + 1) * 128],
                                         ones_col[:],
                                         start=(g == 0), stop=(g == G - 1))

                # rq row (bf16, for broadcast matmul); rk columns (fp32)
                # 1/sqrt(s) = exp(-0.5*ln(s)); batch by ACT table set
                ln_q = spool.tile([1, C], FP32, tag="ln_q")
                nc.scalar.activation(ln_q[:], sqq[:], Ln)
                ln_ks = []
                for dg in range(G):
                    ln_k = spool.tile([128, 1], FP32, tag="ln_k", bufs=G,
                                      name=f"ln_k{dg}")
                    nc.scalar.activation(ln_k[:], sqk_ps[dg][:], Ln)
                    ln_ks.append(ln_k)
                rq_bf = spool.tile([1, C], BF16, tag="rq_bf")
                nc.scalar.activation(rq_bf[:], ln_q[:], Exp, scale=-0.5)
                rk_cols = []
                for dg in range(G):
                    rk = spool.tile([128, 1], FP32, tag="rk", bufs=G,
                                    name=f"rk{dg}")
                    nc.scalar.activation(rk[:], ln_ks[dg][:], Exp, scale=-0.5)
                    rk_cols.append(rk)

                bq_ps = psN.tile([128, C], FP32, tag="bq_ps", name="bq_ps")
                nc.tensor.matmul(bq_ps[:], ones_row[:, 0:128], rq_bf[:],
                                 start=True, stop=True)
                bq = mpool.tile([128, C], FP32, tag="bq", bufs=1)
                nc.scalar.copy(bq[:], bq_ps[:])

            with tc.tile_pool(name="psB", bufs=1, space="PSUM") as psB:
                # ---- G^T per d-group + transforms + A^T ------------------
                at_ps = [psB.tile([128, C], FP32, tag="at", bufs=G,
                                  name=f"at{eg}") for eg in range(G)]
                for dg in range(G):
                    g_ps = psB.tile([128, C], FP32, tag="g_ps", bufs=2,
                                    name=f"g_ps{dg}")
                    for g in range(G):
                        nc.tensor.matmul(g_ps[:],
                                         wk[:, g, dg * 128:(dg + 1) * 128],
                                         m3q[:, g, :],
                                         start=(g == 0), stop=(g == G - 1))
                    # cos = G^T * rq_c * rk_d
                    t1 = mpool.tile([128, C], FP32, tag="t1")
                    nc.vector.tensor_tensor(t1[:], g_ps[:], bq[:], op=MUL)
                    cosd = mpool.tile([128, C], FP32, tag="cosd")
                    nc.vector.tensor_scalar(cosd[:], t1[:], rk_cols[dg][:],
                                            None, op0=MUL)
                    mn = spool.tile([128, 1], FP32, tag="mn")
                    nc.vector.tensor_reduce(mn[:], cosd[:], axis=AX, op=MIN)
                    den = spool.tile([128, 1], FP32, tag="den")
                    nc.vector.tensor_scalar(den[:], mn[:], -1.0, 1.0 + EPS,
                                            op0=MUL, op1=ADD)
                    r = spool.tile([128, 1], FP32, tag="r")
                    nc.vector.reciprocal(r[:], den[:])
                    sv = spool.tile([128, 1], FP32, tag="sv")
                    nc.vector.tensor_scalar(sv[:], r[:], INV_H, 0.0,
                                            op0=MUL, op1=ADD)
                    bv = spool.tile([128, 1], FP32, tag="bv")
                    nc.vector.tensor_scalar(bv[:], r[:], -INV_H, 1.0,
                                            op0=MUL, op1=ADD)
                    e = mpool.tile([128, C], BF16, tag="e")
                    se = spool.tile([128, 1], FP32, tag="se")
                    nc.scalar.activation(e[:], cosd[:], Exp,
                                         bias=bv[:], scale=sv[:],
                                         accum_out=se[:])
                    rd = spool.tile([128, 1], FP32, tag="rd")
                    nc.vector.reciprocal(rd[:], se[:])
                    nc.vector.tensor_scalar(msm[:, dg, :], e[:], rd[:], None,
                                            op0=MUL)
                    # A^T accumulation: A^T = Wv^T-contracted over d
                    for eg in range(G):
                        nc.tensor.matmul(at_ps[eg][:],
                                         wv[:, dg, eg * 128:(eg + 1) * 128],
                                         msm[:, dg, :],
                                         start=(dg == 0), stop=(dg == G - 1))
                for eg in range(G):
                    nc.scalar.copy(at_sb[:, eg, :], at_ps[eg][:])

                # ---- row-L1 sums + final per-row scale -------------------
                s_list = []
                for cg in range(G):
                    s_ps = psB.tile([128, 1], FP32, tag="g_ps", bufs=2,
                                    name=f"s_ps{cg}")
                    for dg in range(G):
                        nc.tensor.matmul(
                            s_ps[:],
                            msm[:, dg, cg * 128:(cg + 1) * 128],
                            ones_col[:], start=(dg == 0), stop=(dg == G - 1))
                    s_list.append(s_ps)
                for cg in range(G):
                    speps = spool.tile([128, 1], FP32, tag="speps")
                    nc.vector.tensor_scalar(speps[:], s_list[cg][:],
                                            EPS, None, op0=ADD)
                    rs = spool.tile([128, 1], FP32, tag="rs")
                    nc.vector.reciprocal(rs[:], speps[:])
                    f = spool.tile([128, 1], FP32, tag="f", bufs=G,
                                   name=f"f{cg}")
                    nc.vector.tensor_tensor(f[:], rs[:], gamma_col[:], op=MUL)
                    fcols.append(f)

            # ---- phase 2: out = A X, scale, residual, store --------------
            with tc.tile_pool(name="ps2", bufs=1, space="PSUM") as ps2:
                xf_tiles = []
                for j in range(NJ):
                    xf_t = p2pool.tile([128, G, 512], FP32, tag="xf", bufs=3,
                                       name=f"xf{j}")
                    nc.sync.dma_start(xf_t[:],
                                      xf_v[:, :, j * 512:(j + 1) * 512])
                    xf_tiles.append(xf_t)
                    if j < 2:
                        continue          # prefetch two chunks ahead
                    _phase2_chunk(nc, tc, ps2, opool, at_sb, xh_t, fcols,
                                  xf_tiles[j - 2], y_v, j - 2)
                for j in (NJ - 2, NJ - 1):
                    _phase2_chunk(nc, tc, ps2, opool, at_sb, xh_t, fcols,
                                  xf_tiles[j], y_v, j)

    nc.compile()
    return nc


def _get_nc():
    if "nc" not in _CACHE:
        _CACHE["nc"] = _build_nc()
    return _CACHE["nc"]


def _make_in_maps(x, Wq, Wk, Wv, gamma):
    xb = np.ascontiguousarray(x.reshape(B, C, N).astype(np.float32))
    xb_h = xb.astype(ml_dtypes.bfloat16)
    xt_h = np.ascontiguousarray(xb_h.transpose(0, 2, 1))
    wqt = np.ascontiguousarray(Wq.T).astype(ml_dtypes.bfloat16)
    wkt = np.ascontiguousarray(Wk.T).astype(ml_dtypes.bfloat16)
    wvo = np.ascontiguousarray(Wv).astype(ml_dtypes.bfloat16)
    gcol = np.full((128, 1), float(np.asarray(gamma).reshape(-1)[0]),
                   np.float32)
    ocol = np.ones((128, 1), ml_dtypes.bfloat16)
    orow = np.ones((1, C), ml_dtypes.bfloat16)
    maps = []
    for i in range(B):
        maps.append({
            "xt": xt_h[i], "xh": xb_h[i], "xf": xb[i],
            "wqt": wqt, "wkt": wkt, "wvo": wvo,
            "gamma_col": gcol, "ones_col": ocol, "ones_row": orow,
        })
    return maps


def kernel(x, Wq, Wk, Wv, gamma, _trace=False, _trace_kwargs=None):
    nc = _get_nc()
    in_maps = _make_in_maps(np.asarray(x), np.asarray(Wq), np.asarray(Wk),
                            np.asarray(Wv), np.asarray(gamma))
    kwargs = {}
    if _trace:
        kwargs = dict(trace=True, **(_trace_kwargs or {}))
    res = bass_utils.run_bass_kernel_spmd(nc, in_maps,
                                          core_ids=list(range(B)), **kwargs)
    y = np.stack([res.results[i]["y"].reshape(C, HH, WW) for i in range(B)])
    if _trace:
        kernel._last_result = res
    return y.astype(np.float32)



# revision 7
# speedup vs baseline: 1.4769x; 1.4769x over previous
"""Trainium2 Bass kernel for the contextual channel-attention transformer block.

Contract: kernel(**inputs) takes the FULL unsharded inputs
(x: (8,512,64,64) f32, Wq/Wk/Wv: (512,512) f32, gamma: (1,) f32) and
returns the FULL (8,512,64,64) f32 output.  Internally the batch is
data-parallel across 8 NeuronCores (one batch element per core).

Per-core algorithm (fp8-e4m3 DoubleRow matmuls, fp32 PSUM accumulation):
  Gx   = X @ X.T                  (fp8 DR, 64 MMs)       Gx/32 -> fp8
  M3q  = Gx Wq^T, M3k = Gx Wk^T   (fp8 DR, 16 MMs)       psum = M3/2
  |Q|^2, |K|^2 = diag(W M3)       (colsum via ones-matmul)
  m3q' = (M3q/128) * rq[c]        (rq = 1/(sqrt8 |Q|), bq row-broadcast)
  G^T  = (16Wk) m3q'              (fp8 DR) = G^T rq / 8
  cos->softmax on G^T[d,c]: rk folded into ACT scale/bias; msm8 = 64*softmax
  A^T  = (16Wv)^T msm8 (fp8 DR), at8 = psum/16 = 64 A^T
  out  = at8 @ X8 (fp8 DR) = 64*out ; y = x_bf16 + (gamma/(64*rowsum)) * out
"""

import os
import sys

for _p in ("/opt/trn_rl_repo", "/root/.axon_site/_ro/trn_rl_repo"):
    if os.path.isdir(_p) and _p not in sys.path:
        sys.path.insert(0, _p)

import ml_dtypes
import numpy as np

import concourse.bass as bass
import concourse.tile as tile
from concourse import bacc, bass_utils, mybir

# Problem constants (hardcoded; kernel.py must be self-contained).
B, C, HH, WW = 8, 512, 64, 64
N = HH * WW          # 4096 spatial positions
G = C // 128         # 4 channel groups of 128
NP = N // 256        # 16 spatial pair-chunks (2x128 for DoubleRow)
NJ = N // 512        # 8 output chunks (512 spatial each)
EPS = 1e-6
INV_H = 4.0          # 1 / 0.25 temperature
LN64 = float(np.log(64.0))
LN256 = float(np.log(256.0))
FP32 = mybir.dt.float32
BF16 = mybir.dt.bfloat16
F8 = mybir.dt.float8e4
F8MAX = 240.0        # TRN FP8_EXP4 max normal
DR = mybir.MatmulPerfMode.DoubleRow

_CACHE = {}


def _build_nc():
    nc = bacc.Bacc("TRN2", target_bir_lowering=False)

    xt_d = nc.dram_tensor("xt", [N, C], F8, kind="ExternalInput")     # X^T fp8
    xh8_d = nc.dram_tensor("xh8", [C, N], F8, kind="ExternalInput")   # X fp8
    xhb_d = nc.dram_tensor("xhb", [C, N], BF16, kind="ExternalInput")
    wq_d = nc.dram_tensor("wq8", [C, C], F8, kind="ExternalInput")    # 16Wq^T
    wk_d = nc.dram_tensor("wk8", [C, C], F8, kind="ExternalInput")    # 16Wk^T
    wv_d = nc.dram_tensor("wv8", [C, C], F8, kind="ExternalInput")    # 16Wv
    gcol_d = nc.dram_tensor("gamma_col", [128, 1], FP32, kind="ExternalInput")
    ocol_d = nc.dram_tensor("ones_col", [128, 1], BF16, kind="ExternalInput")
    o8_d = nc.dram_tensor("ones8", [128, 1], F8, kind="ExternalInput")
    orow_d = nc.dram_tensor("ones_row", [1, C], BF16, kind="ExternalInput")
    y_d = nc.dram_tensor("y", [C, N], BF16, kind="ExternalOutput")

    xt_v = xt_d.ap().rearrange("(i p) c -> p i c", p=128)     # [128, 32, C]
    xh8_v = xh8_d.ap().rearrange("(g p) n -> p g n", p=128)   # [128, G, N]
    xhb_v = xhb_d.ap().rearrange("(g p) n -> p g n", p=128)
    wq_v = wq_d.ap().rearrange("(g p) o -> p g o", p=128)     # [128, G, C]
    wk_v = wk_d.ap().rearrange("(g p) o -> p g o", p=128)
    wv_v = wv_d.ap().rearrange("(g p) o -> p g o", p=128)
    y_v = y_d.ap().rearrange("(g p) n -> p g n", p=128)

    MUL = mybir.AluOpType.mult
    ADD = mybir.AluOpType.add
    MIN = mybir.AluOpType.min
    AX = mybir.AxisListType.X
    Exp = mybir.ActivationFunctionType.Exp
    Ln = mybir.ActivationFunctionType.Ln
    Copy = mybir.ActivationFunctionType.Copy

    with tile.TileContext(nc) as tc:
        with (
            tc.tile_pool(name="consts", bufs=1) as cpool,
            tc.tile_pool(name="weights", bufs=1) as wpool,
            tc.tile_pool(name="xt", bufs=NP) as xtpool,
            tc.tile_pool(name="xh8", bufs=NJ) as xh8pool,
            tc.tile_pool(name="xhb", bufs=NJ) as xhbpool,
            tc.tile_pool(name="gram", bufs=1) as gpool,
            tc.tile_pool(name="small", bufs=2) as spool,
            tc.tile_pool(name="mid", bufs=3) as mpool,
            tc.tile_pool(name="msm", bufs=1) as msmpool,
            tc.tile_pool(name="outs", bufs=4) as opool,
        ):
            # ---- input DMAs (xt first: Gram depends only on it) ----------
            xtp = []
            for i in range(NP):
                t = xtpool.tile([128, 2, C], F8, tag="xt", bufs=NP,
                                name=f"xt{i}")
                nc.sync.dma_start(t[:], xt_v[:, 2 * i:2 * i + 2, :])
                xtp.append(t)

            wq = wpool.tile([128, G, C], F8, tag="wq")
            wk = wpool.tile([128, G, C], F8, tag="wk")
            wv = wpool.tile([128, G, C], F8, tag="wv")
            nc.sync.dma_start(wq[:], wq_v)
            nc.sync.dma_start(wk[:], wk_v)
            nc.sync.dma_start(wv[:], wv_v)

            ones_col = cpool.tile([128, 1], BF16, tag="ones_col")
            nc.sync.dma_start(ones_col[:], ocol_d.ap())
            ones8 = cpool.tile([128, 1], F8, tag="ones8")
            nc.sync.dma_start(ones8[:], o8_d.ap())
            ones_row = cpool.tile([1, C], BF16, tag="ones_row")
            nc.sync.dma_start(ones_row[:], orow_d.ap())
            gamma_col = cpool.tile([128, 1], FP32, tag="gamma_col")
            nc.sync.dma_start(gamma_col[:], gcol_d.ap())

            xh8_t = []
            xhb_t = []
            for j in range(NJ):
                t8 = xh8pool.tile([128, G, 512], F8, tag="xh8", name=f"xh8{j}")
                nc.sync.dma_start(t8[:], xh8_v[:, :, j * 512:(j + 1) * 512])
                xh8_t.append(t8)
            for j in range(NJ):
                tb = xhbpool.tile([128, G, 512], BF16, tag="xhb",
                                  name=f"xhb{j}")
                nc.sync.dma_start(tb[:], xhb_v[:, :, j * 512:(j + 1) * 512])
                xhb_t.append(tb)

            # ---- Gram: Gx = X X^T, fp8 DoubleRow, bank-major -------------
            gx8 = gpool.tile([128, G, C], F8, tag="gx8")
            with tc.tile_pool(name="psG", bufs=1, space="PSUM") as psG:
                for cg in range(G):
                    gx_ps = psG.tile([128, C], FP32, tag="gx", bufs=2,
                                     name=f"gx{cg}")
                    for i in range(NP):
                        nc.tensor.matmul(
                            gx_ps[:], xtp[i][:, :, cg * 128:(cg + 1) * 128],
                            xtp[i][:], start=(i == 0), stop=(i == NP - 1),
                            perf_mode=DR)
                    nc.scalar.activation(gx8[:, cg, :], gx_ps[:], Copy,
                                         scale=1.0 / 32.0)

            m3q16 = gpool.tile([128, G, C], BF16, tag="m3q16")
            m3q8 = gpool.tile([128, G, C], F8, tag="m3q8")
            bq = gpool.tile([128, C], FP32, tag="bq")
            tqs, tks = [], []

            with tc.tile_pool(name="psN", bufs=1, space="PSUM") as psN:
                sqq = psN.tile([1, C], FP32, tag="sqq", name="sqq")

                # ---- M3q = Gx Wq^T (psum = M3q/2) ------------------------
                with tc.tile_pool(name="psQ", bufs=1, space="PSUM") as psQ:
                    for cg in range(G):
                        q_ps = psQ.tile([128, C], FP32, tag="q", bufs=G,
                                        name=f"q{cg}")
                        for t in range(2):
                            nc.tensor.matmul(
                                q_ps[:],
                                gx8[:, 2 * t:2 * t + 2,
                                    cg * 128:(cg + 1) * 128],
                                wq[:, 2 * t:2 * t + 2, :],
                                start=(t == 0), stop=(t == 1), perf_mode=DR)
                        tq = mpool.tile([128, C], BF16, tag="tq", bufs=G,
                                        name=f"tq{cg}")
                        nc.vector.tensor_tensor(tq[:], wq[:, cg, :], q_ps[:],
                                                op=MUL)
                        tqs.append(tq)
                        nc.scalar.activation(m3q16[:, cg, :], q_ps[:], Copy,
                                             scale=1.0 / 64.0)
                    for cg in range(G):
                        nc.tensor.matmul(sqq[:], ones_col[:], tqs[cg][:],
                                         start=(cg == 0), stop=(cg == G - 1))

                # ---- M3k = Gx Wk^T; tk for |K|^2 -------------------------
                with tc.tile_pool(name="psK", bufs=1, space="PSUM") as psK:
                    for cg in range(G):
                        k_ps = psK.tile([128, C], FP32, tag="k", bufs=G,
                                        name=f"k{cg}")
                        for t in range(2):
                            nc.tensor.matmul(
                                k_ps[:],
                                gx8[:, 2 * t:2 * t + 2,
                                    cg * 128:(cg + 1) * 128],
                                wk[:, 2 * t:2 * t + 2, :],
                                start=(t == 0), stop=(t == 1), perf_mode=DR)
                        tk = mpool.tile([128, C], BF16, tag="tk", bufs=G,
                                        name=f"tk{cg}")
                        nc.vector.tensor_tensor(tk[:], wk[:, cg, :], k_ps[:],
                                                op=MUL)
                        tks.append(tk)

                # rq row: sqq = 8|Q|^2 -> rq = 1/sqrt(sqq) (bf16 row)
                ln_q = spool.tile([1, C], FP32, tag="ln_q")
                nc.scalar.activation(ln_q[:], sqq[:], Ln)
                rq_bf = spool.tile([1, C], BF16, tag="rq_bf")
                nc.scalar.activation(rq_bf[:], ln_q[:], Exp, scale=-0.5)

                # sqk columns + bq broadcast
                with tc.tile_pool(name="psS", bufs=1, space="PSUM") as psS:
                    bq_ps = psS.tile([128, C], FP32, tag="bq_ps", name="bq_ps")
                    nc.tensor.matmul(bq_ps[:], ones_row[:, 0:128], rq_bf[:],
                                     start=True, stop=True)
                    nc.scalar.copy(bq[:], bq_ps[:])
                    sqk_ps = [psS.tile([128, 1], FP32, tag="sqk", bufs=G,
                                       name=f"sqk{d}") for d in range(G)]
                    for g in range(G):
                        for dg in range(G):
                            nc.tensor.matmul(sqk_ps[dg][:],
                                             tks[g][:, dg * 128:(dg + 1) * 128],
                                             ones_col[:],
                                             start=(g == 0), stop=(g == G - 1))
                    # z = 64*rk = exp(-0.5*ln(sqk/4096))
                    zs = []
                    for dg in range(G):
                        ln_k = spool.tile([128, 1], FP32, tag="ln_k", bufs=G,
                                          name=f"ln_k{dg}")
                        nc.scalar.activation(ln_k[:], sqk_ps[dg][:], Ln,
                                             scale=1.0 / 4096.0)
                        z = spool.tile([128, 1], FP32, tag="z", bufs=G,
                                       name=f"z{dg}")
                        nc.scalar.activation(z[:], ln_k[:], Exp, scale=-0.5)
                        zs.append(z)

                # m3q8 = (M3q/128) * rq[c]  (fp8, col-scaled)
                for cg in range(G):
                    nc.vector.tensor_tensor(m3q8[:, cg, :], m3q16[:, cg, :],
                                            bq[:], op=MUL)

            # ---- per-dg: G^T -> softmax -> msm8; A^T over dg pairs -------
            msm = msmpool.tile([128, G, C], F8, tag="msm")
            at8 = msmpool.tile([128, G, C], F8, tag="at8")
            fcols = []
            with tc.tile_pool(name="psB", bufs=1, space="PSUM") as psB:
                at_ps = [psB.tile([128, C], FP32, tag="at", bufs=G,
                                  name=f"at{eg}") for eg in range(G)]
                for dg in range(G):
                    g_ps = psB.tile([128, C], FP32, tag="g_ps", bufs=2,
                                    name=f"g_ps{dg}")
                    for t in range(2):
                        nc.tensor.matmul(
                            g_ps[:],
                            wk[:, 2 * t:2 * t + 2, dg * 128:(dg + 1) * 128],
                            m3q8[:, 2 * t:2 * t + 2, :],
                            start=(t == 0), stop=(t == 1), perf_mode=DR)
                    # g_ps = G^T[d,c] * rq[c] / 8 ; cos = g_ps * z[d]/8... :
                    # z = 64 rk: cos = g_ps * z / 8 * ... folded consistently:
                    # min over c, then scalar chain, exp on ACT.
                    mn0 = spool.tile([128, 1], FP32, tag="mn0")
                    nc.vector.tensor_reduce(mn0[:], g_ps[:], axis=AX, op=MIN)
                    mn = spool.tile([128, 1], FP32, tag="mn")
                    nc.vector.tensor_tensor(mn[:], mn0[:], zs[dg][:], op=MUL)
                    # den4 = (1+eps-mn)/4 ; r4 = 4/(1+eps-mn) = r*INV_H
                    den4 = spool.tile([128, 1], FP32, tag="den4")
                    nc.vector.tensor_scalar(den4[:], mn[:], -0.25,
                                            0.25 * (1.0 + EPS),
                                            op0=MUL, op1=ADD)
                    r4 = spool.tile([128, 1], FP32, tag="r4")
                    nc.vector.reciprocal(r4[:], den4[:])
                    sv = spool.tile([128, 1], FP32, tag="sv")
                    nc.vector.tensor_tensor(sv[:], r4[:], zs[dg][:], op=MUL)
                    bv = spool.tile([128, 1], FP32, tag="bv")
                    nc.vector.tensor_scalar(bv[:], r4[:], -1.0, 1.0,
                                            op0=MUL, op1=ADD)
                    e = mpool.tile([128, C], BF16, tag="e")
                    se = spool.tile([128, 1], FP32, tag="se")
                    nc.scalar.activation(e[:], g_ps[:], Exp,
                                         bias=bv[:], scale=sv[:],
                                         accum_out=se[:])
                    se64 = spool.tile([128, 1], FP32, tag="se64")
                    nc.vector.tensor_scalar(se64[:], se[:], 1.0 / 64.0, None,
                                            op0=MUL)
                    rd64 = spool.tile([128, 1], FP32, tag="rd64")
                    nc.vector.reciprocal(rd64[:], se64[:])
                    nc.vector.tensor_scalar(msm[:, dg, :], e[:], rd64[:],
                                            None, op0=MUL)
                    if dg % 2 == 1:
                        t = dg // 2
                        for eg in range(G):
                            nc.tensor.matmul(
                                at_ps[eg][:],
                                wv[:, 2 * t:2 * t + 2,
                                   eg * 128:(eg + 1) * 128],
                                msm[:, 2 * t:2 * t + 2, :],
                                start=(t == 0), stop=(t == 1), perf_mode=DR)
                for eg in range(G):
                    nc.scalar.activation(at8[:, eg, :], at_ps[eg][:], Copy,
                                         scale=1.0 / 16.0)

                # ---- row-L1 sums + final per-row scale -------------------
                s_list = []
                for cg in range(G):
                    s_ps = psB.tile([128, 1], FP32, tag="g_ps", bufs=2,
                                    name=f"s_ps{cg}")
                    for dg in range(G):
                        nc.tensor.matmul(
                            s_ps[:],
                            msm[:, dg, cg * 128:(cg + 1) * 128],
                            ones8[:], start=(dg == 0), stop=(dg == G - 1))
                    s_list.append(s_ps)
                for cg in range(G):
                    speps = spool.tile([128, 1], FP32, tag="speps")
                    nc.vector.tensor_scalar(speps[:], s_list[cg][:],
                                            64.0 * EPS, None, op0=ADD)
                    rs = spool.tile([128, 1], FP32, tag="rs")
                    nc.vector.reciprocal(rs[:], speps[:])
                    f = spool.tile([128, 1], FP32, tag="f", bufs=G,
                                   name=f"f{cg}")
                    nc.vector.tensor_tensor(f[:], rs[:], gamma_col[:], op=MUL)
                    fcols.append(f)

            # ---- phase 2: out = A X (fp8 DR), scale, residual, store -----
            with tc.tile_pool(name="ps2", bufs=1, space="PSUM") as ps2:
                for j in range(NJ):
                    ofin = opool.tile([128, G, 512], BF16, tag="ofin", bufs=3,
                                      name=f"ofin{j}")
                    for cg in range(G):
                        o_ps = ps2.tile([128, 512], FP32, tag="o_ps", bufs=6,
                                        name=f"o_ps{j}_{cg}")
                        for t in range(2):
                            nc.tensor.matmul(
                                o_ps[:],
                                at8[:, 2 * t:2 * t + 2,
                                    cg * 128:(cg + 1) * 128],
                                xh8_t[j][:, 2 * t:2 * t + 2, :],
                                start=(t == 0), stop=(t == 1), perf_mode=DR)
                        osc = opool.tile([128, 512], FP32, tag="osc", bufs=4,
                                         name=f"osc{j}_{cg}")
                        nc.scalar.activation(osc[:], o_ps[:], Copy,
                                             scale=fcols[cg][:])
                        nc.vector.tensor_tensor(ofin[:, cg, :], osc[:],
                                                xhb_t[j][:, cg, :], op=ADD)
                    nc.sync.dma_start(y_v[:, :, j * 512:(j + 1) * 512],
                                      ofin[:])

    nc.compile()
    return nc


def _get_nc():
    if "nc" not in _CACHE:
        _CACHE["nc"] = _build_nc()
    return _CACHE["nc"]


def _f8(a):
    return np.clip(a, -F8MAX, F8MAX).astype(ml_dtypes.float8_e4m3)


def _make_in_maps(x, Wq, Wk, Wv, gamma):
    xb = np.ascontiguousarray(x.reshape(B, C, N).astype(np.float32))
    xh8 = _f8(xb)
    xhb = xb.astype(ml_dtypes.bfloat16)
    xt8 = np.ascontiguousarray(np.clip(xb.transpose(0, 2, 1), -F8MAX, F8MAX)
                               .astype(ml_dtypes.float8_e4m3))
    wq8 = _f8(np.ascontiguousarray(16.0 * Wq.T))
    wk8 = _f8(np.ascontiguousarray(16.0 * Wk.T))
    wv8 = _f8(16.0 * np.asarray(Wv, np.float32))
    gcol = np.full((128, 1), float(np.asarray(gamma).reshape(-1)[0]),
                   np.float32)
    ocol = np.ones((128, 1), ml_dtypes.bfloat16)
    o8 = np.ones((128, 1), ml_dtypes.float8_e4m3)
    orow = np.ones((1, C), ml_dtypes.bfloat16)
    maps = []
    for i in range(B):
        maps.append({
            "xt": xt8[i], "xh8": xh8[i], "xhb": xhb[i],
            "wq8": wq8, "wk8": wk8, "wv8": wv8,
            "gamma_col": gcol, "ones_col": ocol, "ones8": o8,
            "ones_row": orow,
        })
    return maps


def kernel(x, Wq, Wk, Wv, gamma, _trace=False, _trace_kwargs=None):
    nc = _get_nc()
    in_maps = _make_in_maps(np.asarray(x), np.asarray(Wq), np.asarray(Wk),
                            np.asarray(Wv), np.asarray(gamma))
    kwargs = {}
    if _trace:
        kwargs = dict(trace=True, **(_trace_kwargs or {}))
    res = bass_utils.run_bass_kernel_spmd(nc, in_maps,
                                          core_ids=list(range(B)), **kwargs)
    y = np.stack([np.asarray(res.results[i]["y"], np.float32)
                  .reshape(C, HH, WW) for i in range(B)])
    if _trace:
        kernel._last_result = res
    return y


# revision 11
# speedup vs baseline: 1.5161x; 1.0265x over previous
"""Trainium2 Bass kernel for the contextual channel-attention transformer block.

Contract: kernel(**inputs) takes the FULL unsharded inputs
(x: (8,512,64,64) f32, Wq/Wk/Wv: (512,512) f32, gamma: (1,) f32) and
returns the FULL (8,512,64,64) f32 output.  Internally the batch is
data-parallel across 8 NeuronCores (one batch element per core).

Per-core algorithm (fp8-e4m3 DoubleRow matmuls, fp32 PSUM accumulation):
  Gx   = X @ X.T     upper-triangular blocks fp8 DR + 6 PE transposes
  M3q  = Gx Wq^T, M3k = Gx Wk^T   (fp8 DR)   psum = M3/2
  |Q|^2, |K|^2 = diag(W M3) via ones-matmul colsums
  m3q' = (M3q/128) * rq[c]  (rq = 1/(sqrt8 |Q|) row-broadcast)
  G^T  = (16Wk) m3q' (fp8 DR) = G^T rq/8 ; softmax w/ rk folded into
  ACT scale/bias ; msm8 = 64*softmax ; A^T = (16Wv)^T msm8 (fp8 DR)
  out  = at8 @ X8 (fp8 DR) = 64*out ; y = x_bf16 + f*out in one DVE op
"""

import os
import sys

for _p in ("/opt/trn_rl_repo", "/root/.axon_site/_ro/trn_rl_repo"):
    if os.path.isdir(_p) and _p not in sys.path:
        sys.path.insert(0, _p)

import ml_dtypes
import numpy as np

import concourse.bass as bass
import concourse.tile as tile
from concourse import bacc, bass_utils, mybir

B, C, HH, WW = 8, 512, 64, 64
N = HH * WW          # 4096 spatial positions
G = C // 128         # 4 channel groups of 128
NP = N // 256        # 16 spatial pair-chunks (2x128 for DoubleRow)
NJ = N // 512        # 8 output chunks (512 spatial each)
EPS = 1e-6
FP32 = mybir.dt.float32
BF16 = mybir.dt.bfloat16
F8 = mybir.dt.float8e4
F8MAX = 240.0        # TRN FP8_EXP4 max normal
DR = mybir.MatmulPerfMode.DoubleRow
XT_CHUNKS = [(0, 2), (2, 2), (4, 4), (8, 8)]  # (pair_start, npairs)

_CACHE = {}


def _build_nc():
    nc = bacc.Bacc("TRN2", target_bir_lowering=False)

    xt_d = nc.dram_tensor("xt", [N, C], F8, kind="ExternalInput")     # X^T fp8
    xh8_d = nc.dram_tensor("xh8", [C, N], F8, kind="ExternalInput")   # X fp8
    xhb_d = nc.dram_tensor("xhb", [C, N], BF16, kind="ExternalInput")
    wq_d = nc.dram_tensor("wq8", [C, C], F8, kind="ExternalInput")    # 16Wq^T
    wk_d = nc.dram_tensor("wk8", [C, C], F8, kind="ExternalInput")    # 16Wk^T
    wv_d = nc.dram_tensor("wv8", [C, C], F8, kind="ExternalInput")    # 16Wv
    gcol_d = nc.dram_tensor("gamma_col", [128, 1], FP32, kind="ExternalInput")
    id_d = nc.dram_tensor("ident8", [128, 128], F8, kind="ExternalInput")
    y_d = nc.dram_tensor("y", [C, N], BF16, kind="ExternalOutput")

    xt_v = xt_d.ap().rearrange("(i p) c -> p i c", p=128)     # [128, 32, C]
    xh8_v = xh8_d.ap().rearrange("(g p) n -> p g n", p=128)   # [128, G, N]
    xhb_v = xhb_d.ap().rearrange("(g p) n -> p g n", p=128)
    wq_v = wq_d.ap().rearrange("(g p) o -> p g o", p=128)     # [128, G, C]
    wk_v = wk_d.ap().rearrange("(g p) o -> p g o", p=128)
    wv_v = wv_d.ap().rearrange("(g p) o -> p g o", p=128)
    y_v = y_d.ap().rearrange("(g p) n -> p g n", p=128)

    MUL = mybir.AluOpType.mult
    ADD = mybir.AluOpType.add
    MIN = mybir.AluOpType.min
    AX = mybir.AxisListType.X
    Exp = mybir.ActivationFunctionType.Exp
    Ln = mybir.ActivationFunctionType.Ln
    Copy = mybir.ActivationFunctionType.Copy

    with tile.TileContext(nc) as tc:
        with (
            tc.tile_pool(name="consts", bufs=1) as cpool,
            tc.tile_pool(name="weights", bufs=1) as wpool,
            tc.tile_pool(name="xt", bufs=1) as xtpool,
            tc.tile_pool(name="xbig", bufs=1) as xbigpool,
            tc.tile_pool(name="gram", bufs=1) as gpool,
            tc.tile_pool(name="small", bufs=2) as spool,
            tc.tile_pool(name="mid", bufs=3) as mpool,
            tc.tile_pool(name="msm", bufs=1) as msmpool,
            tc.tile_pool(name="outs", bufs=4) as opool,
        ):
            # ---- input DMAs on SP queue (tiny first, then xt) ------------
            ident = cpool.tile([128, 128], F8, tag="ident")
            nc.sync.dma_start(ident[:], id_d.ap())
            gamma_col = cpool.tile([128, 1], FP32, tag="gamma_col")
            nc.sync.dma_start(gamma_col[:], gcol_d.ap())

            xtp = []
            for ci, (p0, np_) in enumerate(XT_CHUNKS):
                t = xtpool.tile([128, 2 * np_, C], F8, tag=f"xt{ci}",
                                name=f"xt{ci}")
                nc.sync.dma_start(t[:], xt_v[:, 2 * p0:2 * (p0 + np_), :])
                xtp.append(t)

            def xt_pair(i):
                """AP [128, 2, C] for spatial pair i."""
                for (p0, np_), t in zip(XT_CHUNKS, xtp):
                    if p0 <= i < p0 + np_:
                        return t[:, 2 * (i - p0):2 * (i - p0) + 2, :]
                raise AssertionError(i)

            wq = wpool.tile([128, G, C], F8, tag="wq")
            wk = wpool.tile([128, G, C], F8, tag="wk")
            wv = wpool.tile([128, G, C], F8, tag="wv")
            nc.sync.dma_start(wq[:], wq_v)
            nc.sync.dma_start(wk[:], wk_v)
            nc.sync.dma_start(wv[:], wv_v)

            xh8 = xbigpool.tile([128, G, N], F8, tag="xh8")
            nc.sync.dma_start(xh8[:], xh8_v)
            xhb = xbigpool.tile([128, G, N], BF16, tag="xhb")
            nc.sync.dma_start(xhb[:], xhb_v)

            # ---- on-device constants + PE warmup -------------------------
            ones_col = cpool.tile([128, 1], BF16, tag="ones_col")
            nc.vector.memset(ones_col[:], 1.0)
            ones8 = cpool.tile([128, 1], F8, tag="ones8")
            nc.vector.memset(ones8[:], 1.0)
            ones_row = cpool.tile([1, C], BF16, tag="ones_row")
            nc.vector.memset(ones_row[:], 1.0)
            wrm = cpool.tile([128, C], BF16, tag="wrm")
            nc.vector.memset(wrm[:], 0.001)

            # ---- Gram: Gx = X X^T upper blocks, fp8 DR, + transposes -----
            gx8 = gpool.tile([128, G, C], F8, tag="gx8")
            with tc.tile_pool(name="psG", bufs=1, space="PSUM") as psG:
                scr = psG.tile([128, 512], FP32, tag="scr", name="warm")
                for i in range(12):
                    nc.tensor.matmul(scr[:], wrm[:, 0:128], wrm[:],
                                     start=(i == 0), stop=(i == 11))
                for cg in range(G):
                    gx_ps = psG.tile([128, C], FP32, tag="gx", bufs=2,
                                     name=f"gx{cg}")
                    for i in range(NP):
                        nc.tensor.matmul(
                            gx_ps[:, cg * 128:],
                            xt_pair(i)[:, :, cg * 128:(cg + 1) * 128],
                            xt_pair(i)[:, :, cg * 128:],
                            start=(i == 0), stop=(i == NP - 1),
                            perf_mode=DR)
                    nc.scalar.activation(gx8[:, cg, cg * 128:],
                                         gx_ps[:, cg * 128:], Copy,
                                         scale=1.0 / 32.0)
                # lower blocks by PE transpose of the upper ones
                for dg in range(1, G):
                    for cg in range(dg):
                        tp_ps = psG.tile([128, 256], F8, tag="tp", bufs=3,
                                         name=f"tp{dg}_{cg}")
                        nc.tensor.transpose(
                            tp_ps[:, 0:256:2],
                            gx8[:, cg, dg * 128:(dg + 1) * 128],
                            ident[:])
                        nc.scalar.activation(gx8[:, dg, cg * 128:(cg + 1) * 128],
                                             tp_ps[:, 0:256:2], Copy)

            m3q16 = gpool.tile([128, G, C], BF16, tag="m3q16")
            m3q8 = gpool.tile([128, G, C], F8, tag="m3q8")
            bq = gpool.tile([128, C], FP32, tag="bq")
            tqs, tks = [], []

            with tc.tile_pool(name="psN", bufs=1, space="PSUM") as psN:
                sqq = psN.tile([1, C], FP32, tag="sqq", name="sqq")

                # ---- M3q = Gx Wq^T (psum = M3q/2) ------------------------
                with tc.tile_pool(name="psQ", bufs=1, space="PSUM") as psQ:
                    for cg in range(G):
                        q_ps = psQ.tile([128, C], FP32, tag="q", bufs=G,
                                        name=f"q{cg}")
                        for t in range(2):
                            nc.tensor.matmul(
                                q_ps[:],
                                gx8[:, 2 * t:2 * t + 2,
                                    cg * 128:(cg + 1) * 128],
                                wq[:, 2 * t:2 * t + 2, :],
                                start=(t == 0), stop=(t == 1), perf_mode=DR)
                        tq = mpool.tile([128, C], BF16, tag="tq", bufs=G,
                                        name=f"tq{cg}")
                        nc.vector.tensor_tensor(tq[:], wq[:, cg, :], q_ps[:],
                                                op=MUL)
                        tqs.append(tq)
                        nc.scalar.activation(m3q16[:, cg, :], q_ps[:], Copy,
                                             scale=1.0 / 64.0)
                    for cg in range(G):
                        nc.tensor.matmul(sqq[:], ones_col[:], tqs[cg][:],
                                         start=(cg == 0), stop=(cg == G - 1))

                # ---- M3k = Gx Wk^T; tk for |K|^2 -------------------------
                with tc.tile_pool(name="psK", bufs=1, space="PSUM") as psK:
                    for cg in range(G):
                        k_ps = psK.tile([128, C], FP32, tag="k", bufs=G,
                                        name=f"k{cg}")
                        for t in range(2):
                            nc.tensor.matmul(
                                k_ps[:],
                                gx8[:, 2 * t:2 * t + 2,
                                    cg * 128:(cg + 1) * 128],
                                wk[:, 2 * t:2 * t + 2, :],
                                start=(t == 0), stop=(t == 1), perf_mode=DR)
                        tk = mpool.tile([128, C], BF16, tag="tk", bufs=G,
                                        name=f"tk{cg}")
                        nc.vector.tensor_tensor(tk[:], wk[:, cg, :], k_ps[:],
                                                op=MUL)
                        tks.append(tk)

                with tc.tile_pool(name="psS", bufs=1, space="PSUM") as psS:
                    sqk_ps = [psS.tile([128, 1], FP32, tag="sqk", bufs=G,
                                       name=f"sqk{d}") for d in range(G)]
                    for g in range(G):
                        for dg in range(G):
                            nc.tensor.matmul(sqk_ps[dg][:],
                                             tks[g][:, dg * 128:(dg + 1) * 128],
                                             ones_col[:],
                                             start=(g == 0), stop=(g == G - 1))
                    # batched ACT tables: all Ln, then all Exp
                    ln_q = spool.tile([1, C], FP32, tag="ln_q")
                    nc.scalar.activation(ln_q[:], sqq[:], Ln)
                    ln_ks = []
                    for dg in range(G):
                        ln_k = spool.tile([128, 1], FP32, tag="ln_k", bufs=G,
                                          name=f"ln_k{dg}")
                        nc.scalar.activation(ln_k[:], sqk_ps[dg][:], Ln,
                                             scale=1.0 / 4096.0)
                        ln_ks.append(ln_k)
                    rq_bf = spool.tile([1, C], BF16, tag="rq_bf")
                    nc.scalar.activation(rq_bf[:], ln_q[:], Exp, scale=-0.5)
                    zs = []
                    for dg in range(G):
                        z = spool.tile([128, 1], FP32, tag="z", bufs=G,
                                       name=f"z{dg}")
                        nc.scalar.activation(z[:], ln_ks[dg][:], Exp,
                                             scale=-0.5)
                        zs.append(z)
                    bq_ps = psS.tile([128, C], FP32, tag="bq_ps", name="bq_ps")
                    nc.tensor.matmul(bq_ps[:], ones_row[:, 0:128], rq_bf[:],
                                     start=True, stop=True)
                    nc.vector.tensor_copy(bq[:], bq_ps[:])

                # m3q8 = (M3q/128) * rq[c]  (fp8, col-scaled)
                for cg in range(G):
                    nc.vector.tensor_tensor(m3q8[:, cg, :], m3q16[:, cg, :],
                                            bq[:], op=MUL)

            # ---- per-dg: G^T -> softmax -> msm8; A^T over dg pairs -------
            msm = msmpool.tile([128, G, C], F8, tag="msm")
            at8 = msmpool.tile([128, G, C], F8, tag="at8")
            fcols = []
            with tc.tile_pool(name="psB", bufs=1, space="PSUM") as psB:
                at_ps = [psB.tile([128, C], FP32, tag="at", bufs=G,
                                  name=f"at{eg}") for eg in range(G)]
                for dg in range(G):
                    g_ps = psB.tile([128, C], FP32, tag="g_ps", bufs=2,
                                    name=f"g_ps{dg}")
                    for t in range(2):
                        nc.tensor.matmul(
                            g_ps[:],
                            wk[:, 2 * t:2 * t + 2, dg * 128:(dg + 1) * 128],
                            m3q8[:, 2 * t:2 * t + 2, :],
                            start=(t == 0), stop=(t == 1), perf_mode=DR)
                    mn0 = spool.tile([128, 1], FP32, tag="mn0")
                    nc.vector.tensor_reduce(mn0[:], g_ps[:], axis=AX, op=MIN)
                    mn = spool.tile([128, 1], FP32, tag="mn")
                    nc.vector.tensor_tensor(mn[:], mn0[:], zs[dg][:], op=MUL)
                    # den4 = (1+eps-mn)/4 ; r4 = 4/(1+eps-mn) = r*INV_H
                    den4 = spool.tile([128, 1], FP32, tag="den4")
                    nc.vector.tensor_scalar(den4[:], mn[:], -0.25,
                                            0.25 * (1.0 + EPS),
                                            op0=MUL, op1=ADD)
                    r4 = spool.tile([128, 1], FP32, tag="r4")
                    nc.vector.reciprocal(r4[:], den4[:])
                    sv = spool.tile([128, 1], FP32, tag="sv")
                    nc.vector.tensor_tensor(sv[:], r4[:], zs[dg][:], op=MUL)
                    bv = spool.tile([128, 1], FP32, tag="bv")
                    nc.vector.tensor_scalar(bv[:], r4[:], -1.0, 1.0,
                                            op0=MUL, op1=ADD)
                    e = mpool.tile([128, C], BF16, tag="e")
                    se = spool.tile([128, 1], FP32, tag="se")
                    nc.scalar.activation(e[:], g_ps[:], Exp,
                                         bias=bv[:], scale=sv[:],
                                         accum_out=se[:])
                    se64 = spool.tile([128, 1], FP32, tag="se64")
                    nc.vector.tensor_scalar(se64[:], se[:], 1.0 / 64.0, None,
                                            op0=MUL)
                    rd64 = spool.tile([128, 1], FP32, tag="rd64")
                    nc.vector.reciprocal(rd64[:], se64[:])
                    nc.vector.tensor_scalar(msm[:, dg, :], e[:], rd64[:],
                                            None, op0=MUL)
                    if dg % 2 == 1:
                        t = dg // 2
                        for eg in range(G):
                            nc.tensor.matmul(
                                at_ps[eg][:],
                                wv[:, 2 * t:2 * t + 2,
                                   eg * 128:(eg + 1) * 128],
                                msm[:, 2 * t:2 * t + 2, :],
                                start=(t == 0), stop=(t == 1), perf_mode=DR)
                # at8 on DVE so the ACT Copy table reload is off the path
                for eg in range(G):
                    nc.vector.tensor_scalar(at8[:, eg, :], at_ps[eg][:],
                                            1.0 / 16.0, None, op0=MUL)

                # ---- row-L1 sums + final per-row scale -------------------
                s_list = []
                for cg in range(G):
                    s_ps = psB.tile([128, 1], FP32, tag="g_ps", bufs=2,
                                    name=f"s_ps{cg}")
                    for dg in range(G):
                        nc.tensor.matmul(
                            s_ps[:],
                            msm[:, dg, cg * 128:(cg + 1) * 128],
                            ones8[:], start=(dg == 0), stop=(dg == G - 1))
                    s_list.append(s_ps)
                for cg in range(G):
                    speps = spool.tile([128, 1], FP32, tag="speps")
                    nc.vector.tensor_scalar(speps[:], s_list[cg][:],
                                            64.0 * EPS, None, op0=ADD)
                    rs = spool.tile([128, 1], FP32, tag="rs")
                    nc.vector.reciprocal(rs[:], speps[:])
                    f = spool.tile([128, 1], FP32, tag="f", bufs=G,
                                   name=f"f{cg}")
                    nc.vector.tensor_tensor(f[:], rs[:], gamma_col[:], op=MUL)
                    fcols.append(f)

            # ---- phase 2: out = A X (fp8 DR); y = x + f*out in one op ----
            with tc.tile_pool(name="ps2", bufs=1, space="PSUM") as ps2:
                for j in range(NJ):
                    ofin = opool.tile([128, G, 512], BF16, tag="ofin", bufs=3,
                                      name=f"ofin{j}")
                    for cg in range(G):
                        o_ps = ps2.tile([128, 512], FP32, tag="o_ps", bufs=6,
                                        name=f"o_ps{j}_{cg}")
                        for t in range(2):
                            nc.tensor.matmul(
                                o_ps[:],
                                at8[:, 2 * t:2 * t + 2,
                                    cg * 128:(cg + 1) * 128],
                                xh8[:, 2 * t:2 * t + 2,
                                    j * 512:(j + 1) * 512],
                                start=(t == 0), stop=(t == 1), perf_mode=DR)
                        nc.vector.scalar_tensor_tensor(
                            ofin[:, cg, :], o_ps[:], fcols[cg][:],
                            xhb[:, cg, j * 512:(j + 1) * 512],
                            op0=MUL, op1=ADD)
                    # y stores on the Activation HWDGE queue (SP is busy)
                    if j == NJ - 1:
                        for cg in range(G):
                            nc.scalar.dma_start(
                                y_v[:, cg, j * 512:(j + 1) * 512],
                                ofin[:, cg, :])
                    else:
                        nc.scalar.dma_start(y_v[:, :, j * 512:(j + 1) * 512],
                                            ofin[:])

    nc.compile()
    return nc


def _get_nc():
    if "nc" not in _CACHE:
        _CACHE["nc"] = _build_nc()
    return _CACHE["nc"]


def _f8(a):
    return np.clip(a, -F8MAX, F8MAX).astype(ml_dtypes.float8_e4m3)


def _make_in_maps(x, Wq, Wk, Wv, gamma):
    xb = np.ascontiguousarray(x.reshape(B, C, N).astype(np.float32))
    xh8 = _f8(xb)
    xhb = xb.astype(ml_dtypes.bfloat16)
    xt8 = np.ascontiguousarray(np.clip(xb.transpose(0, 2, 1), -F8MAX, F8MAX)
                               .astype(ml_dtypes.float8_e4m3))
    wq8 = _f8(np.ascontiguousarray(16.0 * Wq.T))
    wk8 = _f8(np.ascontiguousarray(16.0 * Wk.T))
    wv8 = _f8(16.0 * np.asarray(Wv, np.float32))
    gcol = np.full((128, 1), float(np.asarray(gamma).reshape(-1)[0]),
                   np.float32)
    ident = np.eye(128, dtype=ml_dtypes.float8_e4m3)
    maps = []
    for i in range(B):
        maps.append({
            "xt": xt8[i], "xh8": xh8[i], "xhb": xhb[i],
            "wq8": wq8, "wk8": wk8, "wv8": wv8,
            "gamma_col": gcol, "ident8": ident,
        })
    return maps


def kernel(x, Wq, Wk, Wv, gamma, _trace=False, _trace_kwargs=None):
    nc = _get_nc()
    in_maps = _make_in_maps(np.asarray(x), np.asarray(Wq), np.asarray(Wk),
                            np.asarray(Wv), np.asarray(gamma))
    kwargs = {}
    if _trace:
        kwargs = dict(trace=True, **(_trace_kwargs or {}))
    res = bass_utils.run_bass_kernel_spmd(nc, in_maps,
                                          core_ids=list(range(B)), **kwargs)
    y = np.stack([np.asarray(res.results[i]["y"], np.float32)
                  .reshape(C, HH, WW) for i in range(B)])
    if _trace:
        kernel._last_result = res
    return y
